# revision 6
# baseline (speedup 1.0000x reference)
"""Trainium2 kernel for nn_Loss_HF_86079734546730.

Strategy (8 NeuronCores, SPMD, no collectives):
  - Shard the two [64,3,512,512] inputs spatially over H: core k gets raw
    rows [64k, 64k+64) => shard [64, 3, 64, 512] per tensor (48 MiB/core).
  - DMA: 1 MiB loads (4 batch-pairs) on the sync-engine HWDGE queue
    (hardware descriptor generation; the software DGE path costs ~1us fixed
    overhead per DMA on the Pool engine and serializes the whole load).
  - Band build, per (tensor, channel, batch-pair j, 128-wide wb chunk):
    two fp32r matmuls accumulate into one PSUM region:
      psum += xe^T [Wd | Ws | 0] + xo^T [-Wd | Ws | 0]
    where xe/xo are the even/odd raw-w columns ([128=(2b x 64h), 128 wb]
    strided stationaries) and W holds the vertical Haar combos. PSUM then
    holds the three bands transposed to [wb(spatial) x 192 band-cols]:
    the horizontal Haar pass rides the PSUM accumulation, the vertical
    pass rides W, and fp32r keeps full input precision at bf16 speed
    (moving dim 256).
  - PSUM->SBUF: two j's share one PSUM bank; a single strided copy
    (alternating scalar/vector engines) casts bf16 into the band buffer
    [128 wb, 6176 = (192 band cols + ones)x32 hb].
  - Gram, per (tensor, channel): PE contracts the band buffer over spatial
    (wb partitions x 32 hb column-groups x 2 wb-chunks) into PSUM fp32.
    Symmetric trim: chunk0 = rows 0:128 x cols 0:193, chunk1 = rows
    128:193 x cols 128:193 only; host mirrors. The ones column makes row/
    col 192 the per-band sums, so means/stds reconstruct on host.
  - Host (float64): sum partial Grams over cores, rebuild per-(b,c,band)
    mean/std, expand the normalized-feature Gram algebraically,
    cosine-sim, softmax, KL.
"""

import numpy as np

B, C, H, W = 64, 3, 512, 512
NCORES = 8
HSH = H // NCORES          # 64 raw rows per core
NJ = B // 2                # 32 batch pairs
WB = W // 2                # 256 band cols
EPS_STD = 1e-5
EPS_COS = 1e-8
EPS_P = 1e-8

NBCOL = 6176               # (192 band cols + 1 ones col) x 32 hb

_CACHE = {}


def _make_w():
    """[128, 512] fp32 moving operands for the band-build matmuls.

    Rows: (b in 2) x (h in 64)  [the partition layout of raw tiles]
    cols 0:128   Wd: (b' in 2) x (band' in {lh,hh}) x (hb in 32)
    cols 128:192 Ws: (b' in 2) x (hb in 32)  [hl band]
    cols 192:256 zero pad (fp32r needs moving >= 256 for full rate)
    cols 256:512 = [-Wd | Ws | 0]  (applied to the odd-w stationary)
    """
    wd = np.zeros((128, 128), np.float32)
    ws = np.zeros((128, 64), np.float32)
    for bl in range(2):
        for hb in range(32):
            r0 = bl * 64 + 2 * hb
            r1 = r0 + 1
            wd[r0, bl * 64 + hb] = 0.5       # lh = (hd[2hb]+hd[2hb+1])/2
            wd[r1, bl * 64 + hb] = 0.5
            wd[r0, bl * 64 + 32 + hb] = -0.5  # hh = (-hd[2hb]+hd[2hb+1])/2
            wd[r1, bl * 64 + 32 + hb] = 0.5
            ws[r0, bl * 32 + hb] = 0.5        # hl = (hs[2hb]-hs[2hb+1])/2
            ws[r1, bl * 32 + hb] = -0.5
    w = np.zeros((128, 512), np.float32)
    w[:, 0:128] = wd
    w[:, 128:192] = ws
    w[:, 256:384] = -wd
    w[:, 384:448] = ws
    return w


def _band_col_map():
    """Map band-buffer column g in [0,192) -> (batch, band).

    Pair j occupies cols [192j, 192j+192) in W's column order:
      b0lh, b0hh, b1lh, b1hh, b0hl, b1hl   (x32 hb each)
    Band buffer column index = bandcol*32 + hb with bandcol = 6j + local.
    """
    col_batch = np.zeros(192, np.int64)
    col_band = np.zeros(192, np.int64)
    for j in range(NJ):
        loc = [(0, 0), (0, 1), (1, 0), (1, 1), (0, 2), (1, 2)]
        for li, (bb, band) in enumerate(loc):
            g = 6 * j + li
            col_batch[g] = 2 * j + bb
            col_band[g] = band
    return col_batch, col_band


def _build_nc():
    import concourse.mybir as mybir
    import concourse.tile as tile
    from concourse import bacc

    f32 = mybir.dt.float32
    f32r = mybir.dt.float32r
    bf16 = mybir.dt.bfloat16

    nc = bacc.Bacc()
    za = nc.declare_dram_parameter("za", [B, C, HSH, W], f32r, isOutput=False)
    zs = nc.declare_dram_parameter("zs", [B, C, HSH, W], f32r, isOutput=False)
    wmat = nc.declare_dram_parameter("wmat", [128, 512], f32r, isOutput=False)
    g0 = nc.declare_dram_parameter("G0", [2, C, 128, 193], f32, isOutput=True)
    g1 = nc.declare_dram_parameter("G1", [2, C, 65, 65], f32, isOutput=True)
    zz = [za, zs]

    with tile.TileContext(nc) as tc:
        with (
            tc.tile_pool(name="wconst", bufs=1) as w_pool,
            tc.tile_pool(name="raw", bufs=6) as raw_pool,
            tc.tile_pool(name="bands", bufs=2) as band_pool,
            tc.tile_pool(name="stage", bufs=2) as stage_pool,
            tc.tile_pool(name="pband", bufs=4, space="PSUM") as pb_pool,
            tc.tile_pool(name="pgram", bufs=1, space="PSUM") as pg_pool,
        ):
            w_t = w_pool.tile([128, 512], f32r, tag="wmat")
            nc.sync.dma_start(w_t[:], wmat[:])
            w_r = w_t[:]

            for c in range(C):
                bufs = {}
                for t in range(2):
                    for wbc in range(2):
                        bb = band_pool.tile([128, NBCOL], bf16, tag=f"bb{t}{wbc}")
                        nc.gpsimd.memset(bb[:, 6144:6176], 1.0)
                        bufs[(t, wbc)] = bb

                for t in range(2):
                    pb = {}
                    for j in range(NJ):
                        half = j % 2
                        raw = raw_pool.tile([128, 512], f32r, tag="raw")
                        eng = nc.sync if half == 0 else nc.scalar
                        eng.dma_start(raw[:], zz[t][2 * j : 2 * j + 2, c])
                        xv = raw[:].rearrange("p (w two) -> p w two", two=2)
                        for wbc in range(2):
                            if half == 0:
                                pb[wbc] = pb_pool.tile(
                                    [128, 512],
                                    f32,
                                    tag="pband",
                                    name=f"pband{wbc}",
                                )
                            xe = xv[:, 128 * wbc : 128 * (wbc + 1), 0]
                            xo = xv[:, 128 * wbc : 128 * (wbc + 1), 1]
                            out = pb[wbc][:, 256 * half : 256 * half + 256]
                            nc.tensor.matmul(
                                out, xe, w_r[:, 0:256], start=True, stop=False
                            )
                            nc.tensor.matmul(
                                out, xo, w_r[:, 256:512], start=False, stop=True
                            )
                        if half == 1:
                            for wbc in range(2):
                                src = pb[wbc][:].rearrange(
                                    "p (j2 g) -> p j2 g", j2=2
                                )[:, :, 0:192]
                                dst = bufs[(t, wbc)][
                                    :, 192 * (j - 1) : 192 * (j - 1) + 384
                                ]
                                nc.vector.tensor_copy(dst, src)

                for t in range(2):
                    pg0 = pg_pool.tile([128, 193], f32, tag=f"pg{t}0")
                    pg1 = pg_pool.tile([128, 65], f32, tag=f"pg{t}1")
                    for wbc in range(2):
                        bb3 = bufs[(t, wbc)][:].rearrange(
                            "p (col hb) -> p col hb", hb=32
                        )
                        for hb in range(32):
                            st_ = wbc == 0 and hb == 0
                            sp = wbc == 1 and hb == 31
                            nc.tensor.matmul(
                                pg0[:, :],
                                bb3[:, 0:128, hb],
                                bb3[:, 0:193, hb],
                                start=st_,
                                stop=sp,
                            )
                            nc.tensor.matmul(
                                pg1[:65, :],
                                bb3[:, 128:193, hb],
                                bb3[:, 128:193, hb],
                                start=st_,
                                stop=sp,
                            )
                    st0 = stage_pool.tile([128, 193], f32, tag="st0")
                    st1 = stage_pool.tile([128, 65], f32, tag="st1")
                    nc.vector.tensor_copy(st0[:], pg0[:])
                    nc.vector.tensor_copy(st1[:65, :], pg1[:65, :])
                    nc.sync.dma_start(g0[t, c], st0[:])
                    nc.sync.dma_start(g1[t, c], st1[:65, :])
    if not nc.is_finalized():
        nc.finalize()
    return nc


def _get_nc():
    if "nc" not in _CACHE:
        _CACHE["nc"] = _build_nc()
    return _CACHE["nc"]


def _in_maps(z_ada, z_sou):
    wm = _make_w()
    maps = []
    for k in range(NCORES):
        sl = slice(HSH * k, HSH * (k + 1))
        maps.append(
            {
                "za": np.ascontiguousarray(z_ada[:, :, sl, :]),
                "zs": np.ascontiguousarray(z_sou[:, :, sl, :]),
                "wmat": wm,
            }
        )
    return maps


def _host_finish(g_parts):
    """g_parts: list of per-core (G0 [2,3,128,193], G1 [2,3,65,65]) fp32."""
    s0 = np.zeros((2, C, 128, 193), np.float64)
    s1 = np.zeros((2, C, 65, 65), np.float64)
    for a0, a1 in g_parts:
        s0 += np.asarray(a0, np.float64)
        s1 += np.asarray(a1, np.float64)

    col_batch, _ = _band_col_map()
    S = float(s1[0, 0, 64, 64])

    P = np.zeros((2, B, B), np.float64)
    Bm = np.zeros((192, B), np.float64)
    Bm[np.arange(192), col_batch] = 1.0
    for t in range(2):
        for c in range(C):
            full = np.zeros((193, 193), np.float64)
            full[0:128, :] = s0[t, c]
            full[128:193, 128:193] = s1[t, c]
            full[128:193, 0:128] = s0[t, c][:, 128:193].T
            M = full[:192, :192]
            Tv = full[192, :192]
            mu = Tv / S
            var = (np.diag(M) - Tv * Tv / S) / (S - 1.0)
            sig = np.sqrt(np.maximum(var, 0.0))
            alpha = 1.0 / (3.0 * (sig + EPS_STD))
            Mc = M - np.outer(mu, Tv) - np.outer(Tv, mu) + S * np.outer(mu, mu)
            Ms = (alpha[:, None] * Mc) * alpha[None, :]
            P[t] += Bm.T @ Ms @ Bm

    sims = []
    for t in range(2):
        r = np.sqrt(np.maximum(np.diag(P[t]), 0.0))
        rc = np.maximum(r, EPS_COS)
        sims.append(P[t] / np.outer(rc, rc))

    def softmax_offdiag(sim):
        m = sim.copy()
        np.fill_diagonal(m, -np.inf)
        mx = m.max(axis=1, keepdims=True)
        e = np.exp(m - mx)
        return e / e.sum(axis=1, keepdims=True)

    p_ada = softmax_offdiag(sims[0]) + EPS_P
    p_sou = softmax_offdiag(sims[1]) + EPS_P
    kl = np.sum(p_sou * (np.log(p_sou) - np.log(p_ada))) / B
    return np.float32(kl)


def kernel(z_ada, z_sou):
    from concourse.bass_utils import run_bass_kernel_spmd

    z_ada = np.asarray(z_ada, np.float32)
    z_sou = np.asarray(z_sou, np.float32)
    nc = _get_nc()
    res = run_bass_kernel_spmd(nc, _in_maps(z_ada, z_sou), list(range(NCORES)))
    g_parts = [
        (res.results[k]["G0"], res.results[k]["G1"]) for k in range(NCORES)
    ]
    return _host_finish(g_parts)


# revision 7
# speedup vs baseline: 3.0011x; 3.0011x over previous
"""Trainium2 kernel for nn_Loss_HF_86079734546730.

Strategy (8 NeuronCores, SPMD, no collectives):
  - Shard the two [64,3,512,512] inputs spatially over H: core k gets raw
    rows [64k, 64k+64) => shard [64, 3, 64, 512] per tensor (48 MiB/core).
  - DMA: 12 loads of 4 MiB each (32 batches x one channel), SBUF layout
    [128 = (b32 x qq4), 8192 = (16 h-rows x 512 w)] fp32 -> 32 KiB
    contiguous per partition. 32 KiB descriptors run all 16 DMA engines at
    full rate (~334 GB/s measured); 2 KiB descriptors cap at ~100 GB/s.
  - Vertical Haar (DVE/Pool split): vs/vd[p, (rb8, w512)] = x[p, 2rb*512+w]
    +/- x[p, (2rb+1)*512+w] -- same-partition column blocks, cast to bf16.
  - Band build (PE): per (tile, wbc, rb-half): 24 bf16 matmuls, stationary
    = vs/vd w-parity slice [128, 128wb], moving = +/-0.5*I[128]. The
    horizontal Haar pass rides PSUM accumulation (even-w mm + odd-w mm);
    the +/-0.5*I moving transposes to [wb x (band,b,qq)] while strided
    PSUM out-APs interleave (qq, rb) so the copy out is a single strided
    pattern per half.
  - PSUM->SBUF: one copy (scalar engine) per (tile, wbc, half) casts bf16
    into the band buffer [128 wb, 6176 = (192 band cols + ones) x 32 hb],
    column g = colIdx*32 + hb, colIdx = bt*96 + band*32 + b, hb = qq*8 +
    half*4 + rb.
  - Gram (PE): per (t, c): contract the band buffer over spatial (wb
    partitions x 32 hb column-groups x 2 wb-chunks) into PSUM fp32.
    Symmetric trim: chunk0 = rows 0:128 x cols 0:193, chunk1 = rows
    128:193 x cols 128:193; host mirrors. The ones column makes row/col
    192 the per-band sums, so means/stds reconstruct on host.
  - Host (float64): sum partial Grams over cores, rebuild per-(b,c,band)
    mean/std, expand the normalized-feature Gram algebraically,
    cosine-sim, softmax, KL.
"""

import numpy as np

B, C, H, W = 64, 3, 512, 512
NCORES = 8
HSH = H // NCORES          # 64 raw rows per core
EPS_STD = 1e-5
EPS_COS = 1e-8
EPS_P = 1e-8

BPT = 32                   # batches per raw tile
NT = B // BPT              # 2 raw tiles per (t, c)
NBCOL = 6176               # (192 band cols + 1 ones col) x 32 hb

_CACHE = {}


def _make_w():
    """[128, 256] fp32 moving operands: [+0.5*I | -0.5*I]."""
    w = np.zeros((128, 256), np.float32)
    w[:, 0:128] = 0.5 * np.eye(128, dtype=np.float32)
    w[:, 128:256] = -0.5 * np.eye(128, dtype=np.float32)
    return w


def _col_batch():
    """band-buffer column g = colIdx*32 + hb; colIdx = bt*96 + band*32 + b
    -> batch index bt*32 + b (band order lh, hl, hh; irrelevant to host)."""
    col_batch = np.zeros(192, np.int64)
    for bt in range(NT):
        for band in range(3):
            for b in range(BPT):
                col_batch[bt * 96 + band * 32 + b] = bt * BPT + b
    return col_batch


def _build_nc():
    import concourse.mybir as mybir
    import concourse.tile as tile
    from concourse import bacc

    f32 = mybir.dt.float32
    bf16 = mybir.dt.bfloat16

    nc = bacc.Bacc()
    za = nc.declare_dram_parameter("za", [B, C, HSH, W], f32, isOutput=False)
    zs = nc.declare_dram_parameter("zs", [B, C, HSH, W], f32, isOutput=False)
    wmat = nc.declare_dram_parameter("wmat", [128, 256], bf16, isOutput=False)
    g0 = nc.declare_dram_parameter("G0", [2, C, 128, 193], f32, isOutput=True)
    g1 = nc.declare_dram_parameter("G1", [2, C, 65, 65], f32, isOutput=True)
    zz = [za, zs]

    with tile.TileContext(nc) as tc:
        with (
            tc.tile_pool(name="wconst", bufs=1) as w_pool,
            tc.tile_pool(name="raw", bufs=2) as raw_pool,
            tc.tile_pool(name="vsd", bufs=2) as vsd_pool,
            tc.tile_pool(name="bands", bufs=2) as band_pool,
            tc.tile_pool(name="stage", bufs=2) as stage_pool,
            tc.tile_pool(name="pband", bufs=2, space="PSUM") as pb_pool,
            tc.tile_pool(name="pgram", bufs=1, space="PSUM") as pg_pool,
        ):
            w_t = w_pool.tile([128, 256], bf16, tag="wmat")
            nc.gpsimd.dma_start(w_t[:], wmat[:])
            wp = w_t[:, 0:128]    # +0.5*I
            wm = w_t[:, 128:256]  # -0.5*I

            vert_k = 0
            for c in range(C):
                bufs = {}
                for t in range(2):
                    for wbc in range(2):
                        bb = band_pool.tile([128, NBCOL], bf16, tag=f"bb{t}{wbc}")
                        nc.gpsimd.memset(bb[:, 6144:6176], 1.0)
                        bufs[(t, wbc)] = bb

                for t in range(2):
                    for bt in range(NT):
                        raw = raw_pool.tile([128, 8192], f32, tag="raw")
                        nc.gpsimd.dma_start(
                            raw[:],
                            zz[t][BPT * bt : BPT * (bt + 1), c].rearrange(
                                "b h w -> b (h w)"
                            ),
                        )
                        rv = raw[:].rearrange(
                            "p (rb two w) -> p rb two w", rb=8, two=2
                        )
                        vs = vsd_pool.tile([128, 4096], bf16, tag="vs")
                        vd = vsd_pool.tile([128, 4096], bf16, tag="vd")
                        vsv = vs[:].rearrange("p (rb w) -> p rb w", rb=8)
                        vdv = vd[:].rearrange("p (rb w) -> p rb w", rb=8)
                        # split vertical pass: every 3rd op pair on Pool
                        e0 = nc.gpsimd if vert_k % 3 == 2 else nc.vector
                        e1 = nc.gpsimd if (vert_k + 1) % 3 == 2 else nc.vector
                        vert_k += 2
                        e0.tensor_add(vsv, rv[:, :, 0, :], rv[:, :, 1, :])
                        e1.tensor_sub(vdv, rv[:, :, 0, :], rv[:, :, 1, :])

                        for wbc in range(2):
                            for half in range(2):
                                pb = pb_pool.tile([128, 1536], f32, tag="pband")
                                pbv = pb[:].rearrange(
                                    "p (band b qq rb) -> p band rb b qq",
                                    band=3, b=BPT, qq=4, rb=4,
                                )
                                for rb4 in range(4):
                                    rb = half * 4 + rb4
                                    sv = vs[
                                        :,
                                        rb * 512 + 256 * wbc : rb * 512
                                        + 256 * wbc
                                        + 256,
                                    ].rearrange("p (wb two) -> p wb two", two=2)
                                    sd = vd[
                                        :,
                                        rb * 512 + 256 * wbc : rb * 512
                                        + 256 * wbc
                                        + 256,
                                    ].rearrange("p (wb two) -> p wb two", two=2)
                                    # lh = +0.5 vs_e - 0.5 vs_o
                                    nc.tensor.matmul(
                                        pbv[:, 0, rb4], sv[:, :, 0], wp,
                                        start=True, stop=False,
                                    )
                                    nc.tensor.matmul(
                                        pbv[:, 0, rb4], sv[:, :, 1], wm,
                                        start=False, stop=True,
                                    )
                                    # hl = +0.5 vd_e + 0.5 vd_o
                                    nc.tensor.matmul(
                                        pbv[:, 1, rb4], sd[:, :, 0], wp,
                                        start=True, stop=False,
                                    )
                                    nc.tensor.matmul(
                                        pbv[:, 1, rb4], sd[:, :, 1], wp,
                                        start=False, stop=True,
                                    )
                                    # hh = -0.5 vd_e + 0.5 vd_o
                                    nc.tensor.matmul(
                                        pbv[:, 2, rb4], sd[:, :, 0], wm,
                                        start=True, stop=False,
                                    )
                                    nc.tensor.matmul(
                                        pbv[:, 2, rb4], sd[:, :, 1], wp,
                                        start=False, stop=True,
                                    )
                                # one strided copy: psum (band b qq rb) ->
                                # bb g = (bt*96+band*32+b)*32 + qq*8+half*4+rb
                                src = pb[:].rearrange(
                                    "p (bb qq rb) -> p bb qq rb", bb=96, qq=4, rb=4
                                )
                                dst = bufs[(t, wbc)][
                                    :, bt * 3072 : bt * 3072 + 3072
                                ].rearrange(
                                    "p (bb qq h2 rb) -> p h2 bb qq rb",
                                    bb=96, qq=4, h2=2, rb=4,
                                )[:, half]
                                nc.scalar.activation(
                                    dst, src, mybir.ActivationFunctionType.Copy
                                )

                for t in range(2):
                    pg = pg_pool.tile([128, 258], f32, tag=f"pg{t}")
                    for wbc in range(2):
                        bb3 = bufs[(t, wbc)][:].rearrange(
                            "p (col hb) -> p col hb", hb=32
                        )
                        for hb in range(32):
                            st_ = wbc == 0 and hb == 0
                            sp = wbc == 1 and hb == 31
                            nc.tensor.matmul(
                                pg[:, 0:193],
                                bb3[:, 0:128, hb],
                                bb3[:, 0:193, hb],
                                start=st_, stop=sp,
                            )
                            nc.tensor.matmul(
                                pg[:65, 193:258],
                                bb3[:, 128:193, hb],
                                bb3[:, 128:193, hb],
                                start=st_, stop=sp,
                            )
                    st0 = stage_pool.tile([128, 193], f32, tag="st0")
                    st1 = stage_pool.tile([128, 65], f32, tag="st1")
                    nc.vector.tensor_copy(st0[:], pg[:, 0:193])
                    nc.vector.tensor_copy(st1[:65, :], pg[:65, 193:258])
                    nc.gpsimd.dma_start(g0[t, c], st0[:])
                    nc.gpsimd.dma_start(g1[t, c], st1[:65, :])
    if not nc.is_finalized():
        nc.finalize()
    return nc


def _get_nc():
    if "nc" not in _CACHE:
        _CACHE["nc"] = _build_nc()
    return _CACHE["nc"]


def _in_maps(z_ada, z_sou):
    import ml_dtypes

    wm = _make_w().astype(ml_dtypes.bfloat16)
    maps = []
    for k in range(NCORES):
        sl = slice(HSH * k, HSH * (k + 1))
        maps.append(
            {
                "za": np.ascontiguousarray(z_ada[:, :, sl, :]),
                "zs": np.ascontiguousarray(z_sou[:, :, sl, :]),
                "wmat": wm,
            }
        )
    return maps


def _host_finish(g_parts):
    """g_parts: list of per-core (G0 [2,3,128,193], G1 [2,3,65,65]) fp32."""
    s0 = np.zeros((2, C, 128, 193), np.float64)
    s1 = np.zeros((2, C, 65, 65), np.float64)
    for a0, a1 in g_parts:
        s0 += np.asarray(a0, np.float64)
        s1 += np.asarray(a1, np.float64)

    col_batch = _col_batch()
    S = float(s1[0, 0, 64, 64])

    P = np.zeros((2, B, B), np.float64)
    Bm = np.zeros((192, B), np.float64)
    Bm[np.arange(192), col_batch] = 1.0
    for t in range(2):
        for c in range(C):
            full = np.zeros((193, 193), np.float64)
            full[0:128, :] = s0[t, c]
            full[128:193, 128:193] = s1[t, c]
            full[128:193, 0:128] = s0[t, c][:, 128:193].T
            M = full[:192, :192]
            Tv = full[192, :192]
            mu = Tv / S
            var = (np.diag(M) - Tv * Tv / S) / (S - 1.0)
            sig = np.sqrt(np.maximum(var, 0.0))
            alpha = 1.0 / (3.0 * (sig + EPS_STD))
            Mc = M - np.outer(mu, Tv) - np.outer(Tv, mu) + S * np.outer(mu, mu)
            Ms = (alpha[:, None] * Mc) * alpha[None, :]
            P[t] += Bm.T @ Ms @ Bm

    sims = []
    for t in range(2):
        r = np.sqrt(np.maximum(np.diag(P[t]), 0.0))
        rc = np.maximum(r, EPS_COS)
        sims.append(P[t] / np.outer(rc, rc))

    def softmax_offdiag(sim):
        m = sim.copy()
        np.fill_diagonal(m, -np.inf)
        mx = m.max(axis=1, keepdims=True)
        e = np.exp(m - mx)
        return e / e.sum(axis=1, keepdims=True)

    p_ada = softmax_offdiag(sims[0]) + EPS_P
    p_sou = softmax_offdiag(sims[1]) + EPS_P
    kl = np.sum(p_sou * (np.log(p_sou) - np.log(p_ada))) / B
    return np.float32(kl)


def kernel(z_ada, z_sou):
    from concourse.bass_utils import run_bass_kernel_spmd

    z_ada = np.asarray(z_ada, np.float32)
    z_sou = np.asarray(z_sou, np.float32)
    nc = _get_nc()
    res = run_bass_kernel_spmd(nc, _in_maps(z_ada, z_sou), list(range(NCORES)))
    g_parts = [
        (res.results[k]["G0"], res.results[k]["G1"]) for k in range(NCORES)
    ]
    return _host_finish(g_parts)


# revision 8
# speedup vs baseline: 3.3136x; 1.1041x over previous
"""Trainium2 kernel for nn_Loss_HF_86079734546730.

Strategy (8 NeuronCores, SPMD, no collectives):
  - Shard the two [64,3,512,512] inputs spatially over H: core k gets raw
    rows [64k, 64k+64) => shard [64, 3, 64, 512] per tensor (48 MiB/core).
  - DMA: 12 loads of 4 MiB each (32 batches x one channel), SBUF layout
    [128 = (b32 x qq4), 8192 = (16 h-rows x 512 w)] fp32 -> 32 KiB
    contiguous per partition. 32 KiB descriptors run all 16 DMA engines at
    full rate (~334 GB/s measured); 2 KiB descriptors cap at ~100 GB/s.
  - Vertical Haar (DVE/Pool split): vs/vd[p, (rb8, w512)] = x[p, 2rb*512+w]
    +/- x[p, (2rb+1)*512+w] -- same-partition column blocks, cast to bf16.
  - Band build (PE): per (tile, wbc, rb-half): 24 bf16 matmuls, stationary
    = vs/vd w-parity slice [128, 128wb], moving = +/-0.5*I[128]. The
    horizontal Haar pass rides PSUM accumulation (even-w mm + odd-w mm);
    the +/-0.5*I moving transposes to [wb x (band,b,qq)] while strided
    PSUM out-APs interleave (qq, rb) so the copy out is a single strided
    pattern per half.
  - PSUM->SBUF: one copy (scalar engine) per (tile, wbc, half) casts bf16
    into the band buffer [128 wb, 6176 = (192 band cols + ones) x 32 hb],
    column g = colIdx*32 + hb, colIdx = bt*96 + band*32 + b, hb = qq*8 +
    half*4 + rb.
  - Gram (PE): per (t, c): contract the band buffer over spatial (wb
    partitions x 32 hb column-groups x 2 wb-chunks) into PSUM fp32.
    Symmetric trim: chunk0 = rows 0:128 x cols 0:193, chunk1 = rows
    128:193 x cols 128:193; host mirrors. The ones column makes row/col
    192 the per-band sums, so means/stds reconstruct on host.
  - Host (float64): sum partial Grams over cores, rebuild per-(b,c,band)
    mean/std, expand the normalized-feature Gram algebraically,
    cosine-sim, softmax, KL.
"""

import numpy as np

B, C, H, W = 64, 3, 512, 512
NCORES = 8
HSH = H // NCORES          # 64 raw rows per core
EPS_STD = 1e-5
EPS_COS = 1e-8
EPS_P = 1e-8

BPT = 32                   # batches per raw tile
NT = B // BPT              # 2 raw tiles per (t, c)
NBCOL = 6176               # (192 band cols + 1 ones col) x 32 hb

_CACHE = {}


def _make_w():
    """[128, 256] fp32 moving operands: [+0.5*I | -0.5*I]."""
    w = np.zeros((128, 256), np.float32)
    w[:, 0:128] = 0.5 * np.eye(128, dtype=np.float32)
    w[:, 128:256] = -0.5 * np.eye(128, dtype=np.float32)
    return w


def _col_batch():
    """band-buffer column g = colIdx*32 + hb; colIdx = bt*96 + band*32 + b
    -> batch index bt*32 + b (band order lh, hl, hh; irrelevant to host)."""
    col_batch = np.zeros(192, np.int64)
    for bt in range(NT):
        for band in range(3):
            for b in range(BPT):
                col_batch[bt * 96 + band * 32 + b] = bt * BPT + b
    return col_batch


def _build_nc():
    import concourse.mybir as mybir
    import concourse.tile as tile
    from concourse import bacc

    f32 = mybir.dt.float32
    bf16 = mybir.dt.bfloat16

    nc = bacc.Bacc()
    za = nc.declare_dram_parameter("za", [B, C, HSH, W], f32, isOutput=False)
    zs = nc.declare_dram_parameter("zs", [B, C, HSH, W], f32, isOutput=False)
    wmat = nc.declare_dram_parameter("wmat", [128, 256], bf16, isOutput=False)
    g0 = nc.declare_dram_parameter("G0", [2, C, 128, 193], f32, isOutput=True)
    g1 = nc.declare_dram_parameter("G1", [2, C, 65, 65], f32, isOutput=True)
    zz = [za, zs]

    with tile.TileContext(nc) as tc:
        with (
            tc.tile_pool(name="wconst", bufs=1) as w_pool,
            tc.tile_pool(name="raw", bufs=2) as raw_pool,
            tc.tile_pool(name="vsd", bufs=2) as vsd_pool,
            tc.tile_pool(name="bands", bufs=2) as band_pool,
            tc.tile_pool(name="stage", bufs=2) as stage_pool,
            tc.tile_pool(name="pband", bufs=2, space="PSUM") as pb_pool,
            tc.tile_pool(name="pgram", bufs=1, space="PSUM") as pg_pool,
        ):
            w_t = w_pool.tile([128, 256], bf16, tag="wmat")
            nc.gpsimd.dma_start(w_t[:], wmat[:])
            wp = w_t[:, 0:128]    # +0.5*I
            wm = w_t[:, 128:256]  # -0.5*I

            vert_k = 0
            for c in range(C):
                bufs = {}
                for t in range(2):
                    for wbc in range(2):
                        bb = band_pool.tile([128, NBCOL], bf16, tag=f"bb{t}{wbc}")
                        nc.gpsimd.memset(bb[:, 6144:6176], 1.0)
                        bufs[(t, wbc)] = bb

                for t in range(2):
                    for bt in range(NT):
                        raw = raw_pool.tile([128, 8192], f32, tag="raw")
                        nc.gpsimd.dma_start(
                            raw[:],
                            zz[t][BPT * bt : BPT * (bt + 1), c].rearrange(
                                "b h w -> b (h w)"
                            ),
                        )
                        rv = raw[:].rearrange(
                            "p (rb two w) -> p rb two w", rb=8, two=2
                        )
                        vs = vsd_pool.tile([128, 4096], bf16, tag="vs")
                        vd = vsd_pool.tile([128, 4096], bf16, tag="vd")
                        vsv = vs[:].rearrange("p (rb w) -> p rb w", rb=8)
                        vdv = vd[:].rearrange("p (rb w) -> p rb w", rb=8)
                        # split vertical pass: every 3rd op pair on Pool
                        e0 = nc.gpsimd if vert_k % 3 == 2 else nc.vector
                        e1 = nc.gpsimd if (vert_k + 1) % 3 == 2 else nc.vector
                        vert_k += 2
                        e0.tensor_add(vsv, rv[:, :, 0, :], rv[:, :, 1, :])
                        e1.tensor_sub(vdv, rv[:, :, 0, :], rv[:, :, 1, :])

                        for wbc in range(2):
                            for half in range(2):
                                # psum col = rb4*384 + band*128 + b*4 + qq:
                                # contiguous 128-col accumulation groups,
                                # none crossing a 512-col bank boundary
                                pb = pb_pool.tile([128, 1536], f32, tag="pband")
                                for rb4 in range(4):
                                    rb = half * 4 + rb4
                                    base = rb4 * 384
                                    sv = vs[
                                        :,
                                        rb * 512 + 256 * wbc : rb * 512
                                        + 256 * wbc
                                        + 256,
                                    ].rearrange("p (wb two) -> p wb two", two=2)
                                    sd = vd[
                                        :,
                                        rb * 512 + 256 * wbc : rb * 512
                                        + 256 * wbc
                                        + 256,
                                    ].rearrange("p (wb two) -> p wb two", two=2)
                                    o_lh = pb[:, base : base + 128]
                                    o_hl = pb[:, base + 128 : base + 256]
                                    o_hh = pb[:, base + 256 : base + 384]
                                    # lh = +0.5 vs_e - 0.5 vs_o
                                    nc.tensor.matmul(
                                        o_lh, sv[:, :, 0], wp,
                                        start=True, stop=False,
                                    )
                                    nc.tensor.matmul(
                                        o_lh, sv[:, :, 1], wm,
                                        start=False, stop=True,
                                    )
                                    # hl = +0.5 vd_e + 0.5 vd_o
                                    nc.tensor.matmul(
                                        o_hl, sd[:, :, 0], wp,
                                        start=True, stop=False,
                                    )
                                    nc.tensor.matmul(
                                        o_hl, sd[:, :, 1], wp,
                                        start=False, stop=True,
                                    )
                                    # hh = -0.5 vd_e + 0.5 vd_o
                                    nc.tensor.matmul(
                                        o_hh, sd[:, :, 0], wm,
                                        start=True, stop=False,
                                    )
                                    nc.tensor.matmul(
                                        o_hh, sd[:, :, 1], wp,
                                        start=False, stop=True,
                                    )
                                # copies per band: psum (rb4, b, qq) ->
                                # bb g = (bt*96+band*32+b)*32 + hb,
                                # hb = half*16 + rb4*4 + qq
                                src_all = pb[:].rearrange(
                                    "p (rb4 band b qq) -> p band rb4 b qq",
                                    rb4=4, band=3, b=BPT, qq=4,
                                )
                                dst_all = bufs[(t, wbc)][
                                    :, bt * 3072 : bt * 3072 + 3072
                                ].rearrange(
                                    "p (band b h2 rb4 qq) -> p band h2 rb4 b qq",
                                    band=3, b=BPT, h2=2, rb4=4, qq=4,
                                )
                                for band in range(3):
                                    nc.scalar.activation(
                                        dst_all[:, band, half],
                                        src_all[:, band],
                                        mybir.ActivationFunctionType.Copy,
                                    )

                for t in range(2):
                    pg = pg_pool.tile([128, 258], f32, tag=f"pg{t}")
                    for wbc in range(2):
                        bb3 = bufs[(t, wbc)][:].rearrange(
                            "p (col hb) -> p col hb", hb=32
                        )
                        for hb in range(32):
                            st_ = wbc == 0 and hb == 0
                            sp = wbc == 1 and hb == 31
                            nc.tensor.matmul(
                                pg[:, 0:193],
                                bb3[:, 0:128, hb],
                                bb3[:, 0:193, hb],
                                start=st_, stop=sp,
                            )
                            nc.tensor.matmul(
                                pg[:65, 193:258],
                                bb3[:, 128:193, hb],
                                bb3[:, 128:193, hb],
                                start=st_, stop=sp,
                            )
                    st0 = stage_pool.tile([128, 193], f32, tag="st0")
                    st1 = stage_pool.tile([128, 65], f32, tag="st1")
                    nc.vector.tensor_copy(st0[:], pg[:, 0:193])
                    nc.vector.tensor_copy(st1[:65, :], pg[:65, 193:258])
                    nc.gpsimd.dma_start(g0[t, c], st0[:])
                    nc.gpsimd.dma_start(g1[t, c], st1[:65, :])
    if not nc.is_finalized():
        nc.finalize()
    return nc


def _get_nc():
    if "nc" not in _CACHE:
        _CACHE["nc"] = _build_nc()
    return _CACHE["nc"]


def _in_maps(z_ada, z_sou):
    import ml_dtypes

    wm = _make_w().astype(ml_dtypes.bfloat16)
    maps = []
    for k in range(NCORES):
        sl = slice(HSH * k, HSH * (k + 1))
        maps.append(
            {
                "za": np.ascontiguousarray(z_ada[:, :, sl, :]),
                "zs": np.ascontiguousarray(z_sou[:, :, sl, :]),
                "wmat": wm,
            }
        )
    return maps


def _host_finish(g_parts):
    """g_parts: list of per-core (G0 [2,3,128,193], G1 [2,3,65,65]) fp32."""
    s0 = np.zeros((2, C, 128, 193), np.float64)
    s1 = np.zeros((2, C, 65, 65), np.float64)
    for a0, a1 in g_parts:
        s0 += np.asarray(a0, np.float64)
        s1 += np.asarray(a1, np.float64)

    col_batch = _col_batch()
    S = float(s1[0, 0, 64, 64])

    P = np.zeros((2, B, B), np.float64)
    Bm = np.zeros((192, B), np.float64)
    Bm[np.arange(192), col_batch] = 1.0
    for t in range(2):
        for c in range(C):
            full = np.zeros((193, 193), np.float64)
            full[0:128, :] = s0[t, c]
            full[128:193, 128:193] = s1[t, c]
            full[128:193, 0:128] = s0[t, c][:, 128:193].T
            M = full[:192, :192]
            Tv = full[192, :192]
            mu = Tv / S
            var = (np.diag(M) - Tv * Tv / S) / (S - 1.0)
            sig = np.sqrt(np.maximum(var, 0.0))
            alpha = 1.0 / (3.0 * (sig + EPS_STD))
            Mc = M - np.outer(mu, Tv) - np.outer(Tv, mu) + S * np.outer(mu, mu)
            Ms = (alpha[:, None] * Mc) * alpha[None, :]
            P[t] += Bm.T @ Ms @ Bm

    sims = []
    for t in range(2):
        r = np.sqrt(np.maximum(np.diag(P[t]), 0.0))
        rc = np.maximum(r, EPS_COS)
        sims.append(P[t] / np.outer(rc, rc))

    def softmax_offdiag(sim):
        m = sim.copy()
        np.fill_diagonal(m, -np.inf)
        mx = m.max(axis=1, keepdims=True)
        e = np.exp(m - mx)
        return e / e.sum(axis=1, keepdims=True)

    p_ada = softmax_offdiag(sims[0]) + EPS_P
    p_sou = softmax_offdiag(sims[1]) + EPS_P
    kl = np.sum(p_sou * (np.log(p_sou) - np.log(p_ada))) / B
    return np.float32(kl)


def kernel(z_ada, z_sou):
    from concourse.bass_utils import run_bass_kernel_spmd

    z_ada = np.asarray(z_ada, np.float32)
    z_sou = np.asarray(z_sou, np.float32)
    nc = _get_nc()
    res = run_bass_kernel_spmd(nc, _in_maps(z_ada, z_sou), list(range(NCORES)))
    g_parts = [
        (res.results[k]["G0"], res.results[k]["G1"]) for k in range(NCORES)
    ]
    return _host_finish(g_parts)


# revision 10
# speedup vs baseline: 3.3569x; 1.0131x over previous
"""Trainium2 kernel for nn_Loss_HF_86079734546730.

Strategy (8 NeuronCores, SPMD, no collectives):
  - Shard the two [64,3,512,512] inputs spatially over H: core k gets raw
    rows [64k, 64k+64) => shard [64, 3, 64, 512] per tensor (48 MiB/core).
  - DMA: 12 loads of 4 MiB each (32 batches x one channel), SBUF layout
    [128 = (b32 x qq4), 8192 = (16 h-rows x 512 w)] fp32 -> 32 KiB
    contiguous per partition. 32 KiB descriptors run all 16 DMA engines at
    full rate (~334 GB/s measured); 2 KiB descriptors cap at ~100 GB/s.
  - Vertical Haar (DVE/Pool split): vs/vd[p, (rb8, w512)] = x[p, 2rb*512+w]
    +/- x[p, (2rb+1)*512+w] -- same-partition column blocks, cast to bf16.
  - Band build (PE): per (tile, wbc, rb-half): 24 bf16 matmuls, stationary
    = vs/vd w-parity slice [128, 128wb], moving = +/-0.5*I[128]. The
    horizontal Haar pass rides PSUM accumulation (even-w mm + odd-w mm);
    the +/-0.5*I moving transposes to [wb x (band,b,qq)] while strided
    PSUM out-APs interleave (qq, rb) so the copy out is a single strided
    pattern per half.
  - PSUM->SBUF: one copy (scalar engine) per (tile, wbc, half) casts bf16
    into the band buffer [128 wb, 6176 = (192 band cols + ones) x 32 hb],
    column g = colIdx*32 + hb, colIdx = bt*96 + band*32 + b, hb = qq*8 +
    half*4 + rb.
  - Gram (PE): per (t, c): contract the band buffer over spatial (wb
    partitions x 32 hb column-groups x 2 wb-chunks) into PSUM fp32.
    Symmetric trim: chunk0 = rows 0:128 x cols 0:193, chunk1 = rows
    128:193 x cols 128:193; host mirrors. The ones column makes row/col
    192 the per-band sums, so means/stds reconstruct on host.
  - Host (float64): sum partial Grams over cores, rebuild per-(b,c,band)
    mean/std, expand the normalized-feature Gram algebraically,
    cosine-sim, softmax, KL.
"""

import numpy as np

B, C, H, W = 64, 3, 512, 512
NCORES = 8
HSH = H // NCORES          # 64 raw rows per core
EPS_STD = 1e-5
EPS_COS = 1e-8
EPS_P = 1e-8

BPT = 32                   # batches per raw tile
NT = B // BPT              # 2 raw tiles per (t, c)
NBCOL = 6176               # (192 band cols + 1 ones col) x 32 hb

_CACHE = {}


def _make_w():
    """[128, 256] fp32 moving operands: [+0.5*I | -0.5*I]."""
    w = np.zeros((128, 256), np.float32)
    w[:, 0:128] = 0.5 * np.eye(128, dtype=np.float32)
    w[:, 128:256] = -0.5 * np.eye(128, dtype=np.float32)
    return w


def _col_batch():
    """band-buffer column g = colIdx*32 + hb; colIdx = bt*96 + band*32 + b
    -> batch index bt*32 + b (band order lh, hl, hh; irrelevant to host)."""
    col_batch = np.zeros(192, np.int64)
    for bt in range(NT):
        for band in range(3):
            for b in range(BPT):
                col_batch[bt * 96 + band * 32 + b] = bt * BPT + b
    return col_batch


def _build_nc():
    import concourse.mybir as mybir
    import concourse.tile as tile
    from concourse import bacc

    f32 = mybir.dt.float32
    bf16 = mybir.dt.bfloat16

    nc = bacc.Bacc()
    za = nc.declare_dram_parameter("za", [B, C, HSH, W], f32, isOutput=False)
    zs = nc.declare_dram_parameter("zs", [B, C, HSH, W], f32, isOutput=False)
    wmat = nc.declare_dram_parameter("wmat", [128, 256], bf16, isOutput=False)
    g0 = nc.declare_dram_parameter("G0", [2, C, 128, 193], f32, isOutput=True)
    g1 = nc.declare_dram_parameter("G1", [2, C, 65, 65], f32, isOutput=True)
    zz = [za, zs]

    with tile.TileContext(nc) as tc:
        with (
            tc.tile_pool(name="wconst", bufs=1) as w_pool,
            tc.tile_pool(name="raw", bufs=2) as raw_pool,
            tc.tile_pool(name="vsd", bufs=2) as vsd_pool,
            tc.tile_pool(name="bands", bufs=2) as band_pool,
            tc.tile_pool(name="stage", bufs=2) as stage_pool,
            tc.tile_pool(name="pband", bufs=2, space="PSUM") as pb_pool,
            tc.tile_pool(name="pgram", bufs=2, space="PSUM") as pg_pool,
        ):
            w_t = w_pool.tile([128, 256], bf16, tag="wmat")
            nc.gpsimd.dma_start(w_t[:], wmat[:])
            wp = w_t[:, 0:128]    # +0.5*I
            wm = w_t[:, 128:256]  # -0.5*I

            vert_k = 0
            for c in range(C):
                bufs = {}
                for t in range(2):
                    for wbc in range(2):
                        bb = band_pool.tile([128, NBCOL], bf16, tag=f"bb{t}{wbc}")
                        nc.gpsimd.memset(bb[:, 6144:6176], 1.0)
                        bufs[(t, wbc)] = bb

                for t in range(2):
                    for bt in range(NT):
                        raw = raw_pool.tile([128, 8192], f32, tag="raw")
                        nc.gpsimd.dma_start(
                            raw[:],
                            zz[t][BPT * bt : BPT * (bt + 1), c].rearrange(
                                "b h w -> b (h w)"
                            ),
                        )
                        rv = raw[:].rearrange(
                            "p (rb two w) -> p rb two w", rb=8, two=2
                        )
                        vs = vsd_pool.tile([128, 4096], bf16, tag="vs")
                        vd = vsd_pool.tile([128, 4096], bf16, tag="vd")
                        vsv = vs[:].rearrange("p (rb w) -> p rb w", rb=8)
                        vdv = vd[:].rearrange("p (rb w) -> p rb w", rb=8)
                        # split vertical pass: every 3rd op pair on Pool
                        e0 = nc.gpsimd if vert_k % 3 == 2 else nc.vector
                        e1 = nc.gpsimd if (vert_k + 1) % 3 == 2 else nc.vector
                        vert_k += 2
                        e0.tensor_add(vsv, rv[:, :, 0, :], rv[:, :, 1, :])
                        e1.tensor_sub(vdv, rv[:, :, 0, :], rv[:, :, 1, :])

                        for wbc in range(2):
                            for half in range(2):
                                # psum col = rb4*384 + band*128 + b*4 + qq:
                                # contiguous 128-col accumulation groups,
                                # none crossing a 512-col bank boundary
                                pb = pb_pool.tile([128, 1536], f32, tag="pband")
                                for rb4 in range(4):
                                    rb = half * 4 + rb4
                                    base = rb4 * 384
                                    sv = vs[
                                        :,
                                        rb * 512 + 256 * wbc : rb * 512
                                        + 256 * wbc
                                        + 256,
                                    ].rearrange("p (wb two) -> p wb two", two=2)
                                    sd = vd[
                                        :,
                                        rb * 512 + 256 * wbc : rb * 512
                                        + 256 * wbc
                                        + 256,
                                    ].rearrange("p (wb two) -> p wb two", two=2)
                                    o_lh = pb[:, base : base + 128]
                                    o_hl = pb[:, base + 128 : base + 256]
                                    o_hh = pb[:, base + 256 : base + 384]
                                    # lh = +0.5 vs_e - 0.5 vs_o
                                    nc.tensor.matmul(
                                        o_lh, sv[:, :, 0], wp,
                                        start=True, stop=False,
                                    )
                                    nc.tensor.matmul(
                                        o_lh, sv[:, :, 1], wm,
                                        start=False, stop=True,
                                    )
                                    # hl = +0.5 vd_e + 0.5 vd_o
                                    nc.tensor.matmul(
                                        o_hl, sd[:, :, 0], wp,
                                        start=True, stop=False,
                                    )
                                    nc.tensor.matmul(
                                        o_hl, sd[:, :, 1], wp,
                                        start=False, stop=True,
                                    )
                                    # hh = -0.5 vd_e + 0.5 vd_o
                                    nc.tensor.matmul(
                                        o_hh, sd[:, :, 0], wm,
                                        start=True, stop=False,
                                    )
                                    nc.tensor.matmul(
                                        o_hh, sd[:, :, 1], wp,
                                        start=False, stop=True,
                                    )
                                # copies per band: psum (rb4, b, qq) ->
                                # bb g = (bt*96+band*32+b)*32 + hb,
                                # hb = half*16 + rb4*4 + qq
                                src_all = pb[:].rearrange(
                                    "p (rb4 band b qq) -> p band rb4 b qq",
                                    rb4=4, band=3, b=BPT, qq=4,
                                )
                                dst_all = bufs[(t, wbc)][
                                    :, bt * 3072 : bt * 3072 + 3072
                                ].rearrange(
                                    "p (band b h2 rb4 qq) -> p band h2 rb4 b qq",
                                    band=3, b=BPT, h2=2, rb4=4, qq=4,
                                )
                                for band in range(3):
                                    nc.scalar.activation(
                                        dst_all[:, band, half],
                                        src_all[:, band],
                                        mybir.ActivationFunctionType.Copy,
                                    )

                for t in range(2):
                    pg0 = pg_pool.tile([128, 193], f32, tag="pg", name="pg0")
                    for wbc in range(2):
                        bb3 = bufs[(t, wbc)][:].rearrange(
                            "p (col hb) -> p col hb", hb=32
                        )
                        for hb in range(32):
                            nc.tensor.matmul(
                                pg0[:, :],
                                bb3[:, 0:128, hb],
                                bb3[:, 0:193, hb],
                                start=(wbc == 0 and hb == 0),
                                stop=(wbc == 1 and hb == 31),
                            )
                    st0 = stage_pool.tile([128, 193], f32, tag="st0")
                    nc.vector.tensor_copy(st0[:], pg0[:])
                    nc.gpsimd.dma_start(g0[t, c], st0[:])
                    pg1 = pg_pool.tile([128, 193], f32, tag="pg", name="pg1")
                    for wbc in range(2):
                        bb3 = bufs[(t, wbc)][:].rearrange(
                            "p (col hb) -> p col hb", hb=32
                        )
                        for hb in range(32):
                            nc.tensor.matmul(
                                pg1[:65, 0:65],
                                bb3[:, 128:193, hb],
                                bb3[:, 128:193, hb],
                                start=(wbc == 0 and hb == 0),
                                stop=(wbc == 1 and hb == 31),
                            )
                    st1 = stage_pool.tile([128, 65], f32, tag="st1")
                    nc.vector.tensor_copy(st1[:65, :], pg1[:65, 0:65])
                    nc.gpsimd.dma_start(g1[t, c], st1[:65, :])
    if not nc.is_finalized():
        nc.finalize()
    return nc


def _get_nc():
    if "nc" not in _CACHE:
        _CACHE["nc"] = _build_nc()
    return _CACHE["nc"]


def _in_maps(z_ada, z_sou):
    import ml_dtypes

    wm = _make_w().astype(ml_dtypes.bfloat16)
    maps = []
    for k in range(NCORES):
        sl = slice(HSH * k, HSH * (k + 1))
        maps.append(
            {
                "za": np.ascontiguousarray(z_ada[:, :, sl, :]),
                "zs": np.ascontiguousarray(z_sou[:, :, sl, :]),
                "wmat": wm,
            }
        )
    return maps


def _host_finish(g_parts):
    """g_parts: list of per-core (G0 [2,3,128,193], G1 [2,3,65,65]) fp32."""
    s0 = np.zeros((2, C, 128, 193), np.float64)
    s1 = np.zeros((2, C, 65, 65), np.float64)
    for a0, a1 in g_parts:
        s0 += np.asarray(a0, np.float64)
        s1 += np.asarray(a1, np.float64)

    col_batch = _col_batch()
    S = float(s1[0, 0, 64, 64])

    P = np.zeros((2, B, B), np.float64)
    Bm = np.zeros((192, B), np.float64)
    Bm[np.arange(192), col_batch] = 1.0
    for t in range(2):
        for c in range(C):
            full = np.zeros((193, 193), np.float64)
            full[0:128, :] = s0[t, c]
            full[128:193, 128:193] = s1[t, c]
            full[128:193, 0:128] = s0[t, c][:, 128:193].T
            M = full[:192, :192]
            Tv = full[192, :192]
            mu = Tv / S
            var = (np.diag(M) - Tv * Tv / S) / (S - 1.0)
            sig = np.sqrt(np.maximum(var, 0.0))
            alpha = 1.0 / (3.0 * (sig + EPS_STD))
            Mc = M - np.outer(mu, Tv) - np.outer(Tv, mu) + S * np.outer(mu, mu)
            Ms = (alpha[:, None] * Mc) * alpha[None, :]
            P[t] += Bm.T @ Ms @ Bm

    sims = []
    for t in range(2):
        r = np.sqrt(np.maximum(np.diag(P[t]), 0.0))
        rc = np.maximum(r, EPS_COS)
        sims.append(P[t] / np.outer(rc, rc))

    def softmax_offdiag(sim):
        m = sim.copy()
        np.fill_diagonal(m, -np.inf)
        mx = m.max(axis=1, keepdims=True)
        e = np.exp(m - mx)
        return e / e.sum(axis=1, keepdims=True)

    p_ada = softmax_offdiag(sims[0]) + EPS_P
    p_sou = softmax_offdiag(sims[1]) + EPS_P
    kl = np.sum(p_sou * (np.log(p_sou) - np.log(p_ada))) / B
    return np.float32(kl)


def kernel(z_ada, z_sou):
    from concourse.bass_utils import run_bass_kernel_spmd

    z_ada = np.asarray(z_ada, np.float32)
    z_sou = np.asarray(z_sou, np.float32)
    nc = _get_nc()
    res = run_bass_kernel_spmd(nc, _in_maps(z_ada, z_sou), list(range(NCORES)))
    g_parts = [
        (res.results[k]["G0"], res.results[k]["G1"]) for k in range(NCORES)
    ]
    return _host_finish(g_parts)
